# revision 33
# baseline (speedup 1.0000x reference)
"""Ragged segment mean kernel for Trainium2 (8 NeuronCores, data-parallel).

Problem: seq [64, 2048, 1024] f32, begin/end [64] i64.
Output: out[i] = mean(seq[i, begin[i]:end[i], :])  -> [64, 1024] f32.

Strategy: dense-stream architecture. The host concatenates exactly the
segment rows of all samples into one global row stream, cuts it into 8
equal per-core chunks of R rows (R = ceil(total/8) rounded to 128), and
hands each core a contiguous [R, 1024] buffer plus a per-row slot id.
All device-side DMA offsets are then compile-time constants: no runtime
offset registers, no over-read, and per-core load is balanced to the
row.

The seq tensor is declared float32r end-to-end (same 4-byte storage,
host passes raw fp32 bits): the PE's f32r mode rounds operands
internally (~1e-4 rel, far inside the 2e-2 gate) and streams 1
cycle/row instead of fp32's 4, so no exact hi/resid two-pass split is
needed. The DVE expands the slot ids into a one-hot f32r routing mask
(16 is_equal sweeps) while the first data group is in flight; each
128-row chunk is then reduced on the PE as
acc[NSLOT, 512] += mask[128, NSLOT].T @ chunk[128, 512], accumulated
in PSUM across all chunks, scaled by 1/span, and stored. The kernel is
purely HBM-DMA-bound: 2 MiB groups split across both HWDGE rings
sustain ~420 GB/s/core (SDMA port line rate).

A sample whose rows straddle a core boundary becomes one piece per
core; each piece is scaled by the full 1/span on device and the host
adds the partial outputs while scattering back to batch order.
"""

import numpy as np

import concourse.bacc as bacc
import concourse.bass as bass
import concourse.mybir as mybir
import concourse.tile as tile
from concourse.bass_utils import run_bass_kernel_spmd

B, L, D = 64, 2048, 1024
NCORES = 8
FREE = 512              # PSUM bank limit for matmul free dim
NMM = D // FREE         # 2 matmuls per 128-row chunk
GROUP = 512             # bulk rows per dma_start -> 2 MiB
NSLOT_DEFAULT = 16      # routing-mask columns (pieces per core)

_nc_cache = {}


def _group_sizes(R):
    """Split R rows into dma_start group sizes: 2 MiB bulk groups (16 KiB
    contiguous per partition each) with a tapered 256/128-row tail so
    group semaphores arrive at a fine cadence near the end and the PE
    drains right behind the stream. Groups stay smallish because a matmul
    can only start once its whole group's DMA semaphore fires: 4 MiB
    groups were measured to turn the PE into a bursty straggler that
    outlives the stream by >10us."""
    sizes = []
    rem = R
    while rem >= GROUP + 384:
        sizes.append(GROUP)
        rem -= GROUP
    if rem > 384:
        sizes.append(rem - 384)
        rem = 384
    if rem == 384:
        sizes += [256, 128]
    elif rem:
        sizes.append(rem)
    return sizes


def _ring_plan(sizes, mask_rows):
    """Assign each group to one of the two HWDGE rings (0=SP carries the
    mask first, 1=ACT), greedily balancing bytes so both rings finish
    together; a lopsided split leaves one ring streaming alone at half
    the aggregate rate at the end while the PE waits."""
    loads = [mask_rows, 0.0]
    rings = []
    for u in sizes:
        r = 0 if loads[0] <= loads[1] else 1
        rings.append(r)
        loads[r] += u
    return rings


def _build_nc(R, NSLOT):
    nc = bacc.Bacc("TRN2", target_bir_lowering=False)
    f32 = mybir.dt.float32
    f32r = mybir.dt.float32r
    NCH = R // 128
    i32 = mybir.dt.int32
    # seq is declared float32r end-to-end (same 4-byte storage; the host
    # supplies raw fp32 bits). The PE's f32r mode rounds inputs
    # internally (~1e-4 rel) and runs 1 cycle/row instead of fp32's 4;
    # feeding it straight from DMA keeps ACT/DVE nearly idle.
    seqc = nc.dram_tensor("seqc", [R, D], f32r, kind="ExternalInput")
    # per-row slot ids; the 0/1 routing mask is expanded on-device (DVE
    # is_equal) so only NCH*4 bytes/partition ride the DMA ring instead
    # of the NCH*NSLOT*4-byte one-hot mask.
    rowslot = nc.dram_tensor("rowslot", [128, NCH], i32, kind="ExternalInput")
    invc = nc.dram_tensor("invc", [NSLOT, 1], f32, kind="ExternalInput")
    out = nc.dram_tensor("out", [NSLOT, D], f32, kind="ExternalOutput")

    sizes = _group_sizes(R)
    GF = (GROUP // 128) * D  # free size of a full group tile

    with tile.TileContext(nc) as tc:
        with (
            tc.tile_pool(name="const", bufs=1) as cpool,
            tc.tile_pool(name="seqp", bufs=8) as spool,
            tc.tile_pool(name="accp", bufs=1, space="PSUM") as ppool,
            tc.tile_pool(name="resp", bufs=1) as rpool,
        ):
            rs = cpool.tile([128, NCH], i32, tag="rs")
            mt = cpool.tile([128, NCH * NSLOT], f32r)
            iv = cpool.tile([NSLOT, 1], f32)
            iv2 = cpool.tile([NSLOT, 1], f32, tag="iv2")
            warm = ppool.tile([NSLOT, NSLOT], f32, tag="warm")
            acc = ppool.tile([NSLOT, D], f32)

            # Slot ids head the ACT ring (tiny, lands in ~1us) while the
            # SP ring starts the seq stream immediately; the DVE expands
            # them into the one-hot f32r routing mask while the first
            # group is still in flight.
            nc.scalar.dma_start(out=rs[:], in_=rowslot[:])
            nc.scalar.dma_start(out=iv[:], in_=invc[:])
            nc.vector.tensor_copy(out=iv2[:], in_=iv[:])
            mtv = mt[:].rearrange("p (c s) -> p c s", s=NSLOT)
            for s in range(NSLOT):
                nc.vector.tensor_scalar(
                    out=mtv[:, :, s : s + 1],
                    in0=rs[:],
                    scalar1=s,
                    scalar2=None,
                    op0=mybir.AluOpType.is_equal,
                )

            g0 = 0  # stream row base of current group
            ch = 0  # global 128-row chunk counter
            for gi, U in enumerate(sizes):
                J = U // 128
                gf = J * D
                t = spool.tile([128, GF], f32r)
                # group tile[p, j*D+d] holds stream row g0 + p*J + j:
                # partition p reads J*4KiB contiguous from DRAM. Groups
                # are spread over the two HWDGE rings (SP and ACT) so
                # per-dma queue gaps on one ring hide under the other's
                # stream and the SDMA engines never starve.
                src = seqc[g0 : g0 + U, :].rearrange("(p j) d -> p (j d)", p=128)
                ring = nc.sync if gi % 2 == 0 else nc.scalar
                ring.dma_start(out=t[:, 0:gf], in_=src)
                if gi == 0:
                    # warmup matmul consuming only the mask tile: absorbs
                    # the mask-arrival dependency into the PE clock so the
                    # first real matmul waits only on the seq pipeline.
                    nc.tensor.matmul(
                        out=warm[:],
                        lhsT=mt[:, 0:NSLOT],
                        rhs=mt[:, 0:NSLOT],
                        start=True,
                        stop=True,
                    )
                for j in range(J):
                    lhs = mt[:, (ch + j) * NSLOT : (ch + j + 1) * NSLOT]
                    for h in range(NMM):
                        nc.tensor.matmul(
                            out=acc[:, h * FREE : (h + 1) * FREE],
                            lhsT=lhs,
                            rhs=t[:, j * D + h * FREE : j * D + (h + 1) * FREE],
                            start=(ch + j == 0),
                            stop=(ch + j == NCH - 1),
                        )
                ch += J
                g0 += U

            res = rpool.tile([NSLOT, D], f32)
            nc.vector.tensor_scalar_mul(out=res[:], in0=acc[:], scalar1=iv2[:])
            nc.sync.dma_start(out=out[:], in_=res[:])
    nc.compile()
    return nc


def _plan(begin, end):
    """Order samples (big/small interleave to bound pieces per core),
    cut the global segment-row stream into 8 R-row cores, and return
    (R, NSLOT, pieces) with pieces[ci] = [(sample, src_b, src_e, local
    row start), ...]."""
    spans = (end - begin).astype(np.int64)
    desc = np.argsort(-spans, kind="stable")
    order = np.empty(B, dtype=np.int64)
    order[0::2] = desc[: (B + 1) // 2]
    order[1::2] = desc[(B + 1) // 2 :][::-1]
    total = int(spans.sum())
    per_core = -(-total // NCORES)          # ceil(total / 8)
    R = -(-per_core // 128) * 128           # round up to 128 rows

    pieces = [[] for _ in range(NCORES)]
    g = 0  # global stream cursor
    for i in order:
        b, e = int(begin[i]), int(end[i])
        while b < e:
            ci = g // R
            room = (ci + 1) * R - g
            n = min(e - b, room)
            pieces[ci].append((int(i), b, b + n, g - ci * R))
            b += n
            g += n
    nslot = max(NSLOT_DEFAULT, max(len(p) for p in pieces))
    nslot = -(-nslot // 8) * 8
    return R, nslot, pieces


def _make_in_maps(seq, pieces, spans, R, NSLOT):
    NCH = R // 128
    in_maps = []
    for ci in range(NCORES):
        seqc = np.zeros((R, D), dtype=np.float32)
        row_slot = np.full(R, -1, dtype=np.int64)
        inv = np.zeros((NSLOT, 1), dtype=np.float32)
        for s, (i, sb, se, ls) in enumerate(pieces[ci]):
            n = se - sb
            seqc[ls : ls + n] = seq[i, sb:se]
            row_slot[ls : ls + n] = s
            inv[s, 0] = np.float32(1.0 / float(spans[i]))
        rs = np.full((128, NCH), -1, dtype=np.int32)
        g0 = 0
        chb = 0
        for U in _group_sizes(R):
            J = U // 128
            for j in range(J):
                rs[:, chb + j] = row_slot[g0 + np.arange(128) * J + j]
            g0 += U
            chb += J
        in_maps.append({"seqc": seqc, "rowslot": rs, "invc": inv})
    return in_maps


def _axon_reset():
    """Best-effort NeuronCore reset (recovers a device wedged by an
    earlier failed run in the same container)."""
    try:
        import ctypes

        import jax

        jax.devices()
        lib = ctypes.CDLL("/opt/axon/libaxon_pjrt.so")
        lib.axon_reset.restype = ctypes.c_int64
        lib.axon_reset()
    except Exception:
        pass


def _run(seq, begin, end, trace=False):
    seq = np.asarray(seq)
    begin = np.asarray(begin).astype(np.int64)
    end = np.asarray(end).astype(np.int64)
    spans = end - begin
    R, NSLOT, pieces = _plan(begin, end)
    key = (R, NSLOT)
    if key not in _nc_cache:
        _nc_cache[key] = _build_nc(R, NSLOT)
    in_maps = _make_in_maps(seq, pieces, spans, R, NSLOT)
    try:
        res = run_bass_kernel_spmd(
            _nc_cache[key], in_maps, list(range(NCORES)), trace=trace
        )
    except Exception:
        _axon_reset()
        res = run_bass_kernel_spmd(
            _nc_cache[key], in_maps, list(range(NCORES)), trace=trace
        )
    out = np.zeros((B, D), dtype=np.float32)
    for ci in range(NCORES):
        part = res.results[ci]["out"]
        for s, (i, sb, se, ls) in enumerate(pieces[ci]):
            out[i] += part[s]
    return out, res


def kernel(seq, begin, end):
    out, _ = _run(seq, begin, end, trace=False)
    return out


# revision 35
# speedup vs baseline: 1.4291x; 1.4291x over previous
"""Ragged segment mean kernel for Trainium2 (8 NeuronCores, data-parallel).

Problem: seq [64, 2048, 1024] f32, begin/end [64] i64.
Output: out[i] = mean(seq[i, begin[i]:end[i], :])  -> [64, 1024] f32.

Strategy: dense-stream architecture. The host concatenates exactly the
segment rows of all samples into one global row stream, cuts it into 8
equal per-core chunks of R rows (R = ceil(total/8) rounded to 128), and
hands each core a contiguous [R, 1024] buffer plus a per-row slot id.
All device-side DMA offsets are then compile-time constants: no runtime
offset registers, no over-read, and per-core load is balanced to the
row.

The seq tensor is declared float32r end-to-end (same 4-byte storage,
host passes raw fp32 bits): the PE's f32r mode rounds operands
internally (~1e-4 rel, far inside the 2e-2 gate) and streams 1
cycle/row instead of fp32's 4, so no exact hi/resid two-pass split is
needed. The DVE expands the slot ids into a one-hot f32r routing mask
(16 is_equal sweeps) while the first data group is in flight; each
128-row chunk is then reduced on the PE as
acc[NSLOT, 512] += mask[128, NSLOT].T @ chunk[128, 512], accumulated
in PSUM across all chunks, scaled by 1/span, and stored. The kernel is
purely HBM-DMA-bound: 2 MiB groups split across both HWDGE rings
sustain ~420 GB/s/core (SDMA port line rate).

A sample whose rows straddle a core boundary becomes one piece per
core; each piece is scaled by the full 1/span on device and the host
adds the partial outputs while scattering back to batch order.
"""

import ml_dtypes
import numpy as np

import concourse.bacc as bacc
import concourse.bass as bass
import concourse.mybir as mybir
import concourse.tile as tile
from concourse.bass_utils import run_bass_kernel_spmd

B, L, D = 64, 2048, 1024
NCORES = 8
FREE = 512              # PSUM bank limit for matmul free dim
NMM = D // FREE         # 2 matmuls per 128-row chunk
GROUP = 512             # bulk rows per dma_start -> 2 MiB
NSLOT_DEFAULT = 16      # routing-mask columns (pieces per core)

_nc_cache = {}


def _group_sizes(R):
    """Split R rows into dma_start group sizes: 2 MiB bulk groups (16 KiB
    contiguous per partition each) with a tapered 256/128-row tail so
    group semaphores arrive at a fine cadence near the end and the PE
    drains right behind the stream. Groups stay smallish because a matmul
    can only start once its whole group's DMA semaphore fires: 4 MiB
    groups were measured to turn the PE into a bursty straggler that
    outlives the stream by >10us."""
    sizes = []
    rem = R
    while rem >= GROUP + 384:
        sizes.append(GROUP)
        rem -= GROUP
    if rem > 384:
        sizes.append(rem - 384)
        rem = 384
    if rem == 384:
        sizes += [256, 128]
    elif rem:
        sizes.append(rem)
    return sizes


def _ring_plan(sizes, mask_rows):
    """Assign each group to one of the two HWDGE rings (0=SP carries the
    mask first, 1=ACT), greedily balancing bytes so both rings finish
    together; a lopsided split leaves one ring streaming alone at half
    the aggregate rate at the end while the PE waits."""
    loads = [mask_rows, 0.0]
    rings = []
    for u in sizes:
        r = 0 if loads[0] <= loads[1] else 1
        rings.append(r)
        loads[r] += u
    return rings


def _build_nc(R, NSLOT):
    nc = bacc.Bacc("TRN2", target_bir_lowering=False)
    f32 = mybir.dt.float32
    NCH = R // 128
    i32 = mybir.dt.int32
    bf16 = mybir.dt.bfloat16
    # seq is staged by the host as bf16: the 2e-2 accuracy gate leaves
    # ~20x margin over bf16 rounding (~1e-3 on the segment mean), and
    # halving the bytes halves the HBM-bound stream time. bf16 matmuls
    # run 1 cycle/row like f32r.
    seqc = nc.dram_tensor("seqc", [R, D], bf16, kind="ExternalInput")
    # per-row slot ids; the 0/1 routing mask is expanded on-device (DVE
    # is_equal) so only NCH*4 bytes/partition ride the DMA ring instead
    # of the NCH*NSLOT*4-byte one-hot mask.
    rowslot = nc.dram_tensor("rowslot", [128, NCH], i32, kind="ExternalInput")
    invc = nc.dram_tensor("invc", [NSLOT, 1], f32, kind="ExternalInput")
    out = nc.dram_tensor("out", [NSLOT, D], f32, kind="ExternalOutput")

    sizes = _group_sizes(R)
    GF = (GROUP // 128) * D  # free size of a full group tile

    with tile.TileContext(nc) as tc:
        with (
            tc.tile_pool(name="const", bufs=1) as cpool,
            tc.tile_pool(name="seqp", bufs=8) as spool,
            tc.tile_pool(name="accp", bufs=1, space="PSUM") as ppool,
            tc.tile_pool(name="resp", bufs=1) as rpool,
        ):
            rs = cpool.tile([128, NCH], i32, tag="rs")
            mt = cpool.tile([128, NCH * NSLOT], bf16)
            iv = cpool.tile([NSLOT, 1], f32)
            iv2 = cpool.tile([NSLOT, 1], f32, tag="iv2")
            warm = ppool.tile([NSLOT, NSLOT], f32, tag="warm")
            acc = ppool.tile([NSLOT, D], f32)

            # Slot ids head the ACT ring (tiny, lands in ~1us) while the
            # SP ring starts the seq stream immediately; the DVE expands
            # them into the one-hot f32r routing mask while the first
            # group is still in flight.
            nc.scalar.dma_start(out=rs[:], in_=rowslot[:])
            nc.scalar.dma_start(out=iv[:], in_=invc[:])
            nc.vector.tensor_copy(out=iv2[:], in_=iv[:])
            mtv = mt[:].rearrange("p (c s) -> p c s", s=NSLOT)
            for s in range(NSLOT):
                nc.vector.tensor_scalar(
                    out=mtv[:, :, s : s + 1],
                    in0=rs[:],
                    scalar1=s,
                    scalar2=None,
                    op0=mybir.AluOpType.is_equal,
                )

            g0 = 0  # stream row base of current group
            ch = 0  # global 128-row chunk counter
            for gi, U in enumerate(sizes):
                J = U // 128
                gf = J * D
                t = spool.tile([128, GF], bf16)
                # group tile[p, j*D+d] holds stream row g0 + p*J + j:
                # partition p reads J*4KiB contiguous from DRAM. Groups
                # are spread over the two HWDGE rings (SP and ACT) so
                # per-dma queue gaps on one ring hide under the other's
                # stream and the SDMA engines never starve.
                src = seqc[g0 : g0 + U, :].rearrange("(p j) d -> p (j d)", p=128)
                ring = nc.sync if gi % 2 == 0 else nc.scalar
                ring.dma_start(out=t[:, 0:gf], in_=src)
                if gi == 0:
                    # warmup matmul consuming only the mask tile: absorbs
                    # the mask-arrival dependency into the PE clock so the
                    # first real matmul waits only on the seq pipeline.
                    nc.tensor.matmul(
                        out=warm[:],
                        lhsT=mt[:, 0:NSLOT],
                        rhs=mt[:, 0:NSLOT],
                        start=True,
                        stop=True,
                    )
                for j in range(J):
                    lhs = mt[:, (ch + j) * NSLOT : (ch + j + 1) * NSLOT]
                    for h in range(NMM):
                        nc.tensor.matmul(
                            out=acc[:, h * FREE : (h + 1) * FREE],
                            lhsT=lhs,
                            rhs=t[:, j * D + h * FREE : j * D + (h + 1) * FREE],
                            start=(ch + j == 0),
                            stop=(ch + j == NCH - 1),
                        )
                ch += J
                g0 += U

            res = rpool.tile([NSLOT, D], f32)
            nc.vector.tensor_scalar_mul(out=res[:], in0=acc[:], scalar1=iv2[:])
            nc.sync.dma_start(out=out[:], in_=res[:])
    nc.compile()
    return nc


def _plan(begin, end):
    """Order samples (big/small interleave to bound pieces per core),
    cut the global segment-row stream into 8 R-row cores, and return
    (R, NSLOT, pieces) with pieces[ci] = [(sample, src_b, src_e, local
    row start), ...]."""
    spans = (end - begin).astype(np.int64)
    desc = np.argsort(-spans, kind="stable")
    order = np.empty(B, dtype=np.int64)
    order[0::2] = desc[: (B + 1) // 2]
    order[1::2] = desc[(B + 1) // 2 :][::-1]
    total = int(spans.sum())
    per_core = -(-total // NCORES)          # ceil(total / 8)
    R = -(-per_core // 128) * 128           # round up to 128 rows

    pieces = [[] for _ in range(NCORES)]
    g = 0  # global stream cursor
    for i in order:
        b, e = int(begin[i]), int(end[i])
        while b < e:
            ci = g // R
            room = (ci + 1) * R - g
            n = min(e - b, room)
            pieces[ci].append((int(i), b, b + n, g - ci * R))
            b += n
            g += n
    nslot = max(NSLOT_DEFAULT, max(len(p) for p in pieces))
    nslot = -(-nslot // 8) * 8
    return R, nslot, pieces


def _make_in_maps(seq, pieces, spans, R, NSLOT):
    NCH = R // 128
    in_maps = []
    for ci in range(NCORES):
        seqc = np.zeros((R, D), dtype=ml_dtypes.bfloat16)
        row_slot = np.full(R, -1, dtype=np.int64)
        inv = np.zeros((NSLOT, 1), dtype=np.float32)
        for s, (i, sb, se, ls) in enumerate(pieces[ci]):
            n = se - sb
            seqc[ls : ls + n] = seq[i, sb:se]
            row_slot[ls : ls + n] = s
            inv[s, 0] = np.float32(1.0 / float(spans[i]))
        rs = np.full((128, NCH), -1, dtype=np.int32)
        g0 = 0
        chb = 0
        for U in _group_sizes(R):
            J = U // 128
            for j in range(J):
                rs[:, chb + j] = row_slot[g0 + np.arange(128) * J + j]
            g0 += U
            chb += J
        in_maps.append({"seqc": seqc, "rowslot": rs, "invc": inv})
    return in_maps


def _axon_reset():
    """Best-effort NeuronCore reset (recovers a device wedged by an
    earlier failed run in the same container)."""
    try:
        import ctypes

        import jax

        jax.devices()
        lib = ctypes.CDLL("/opt/axon/libaxon_pjrt.so")
        lib.axon_reset.restype = ctypes.c_int64
        lib.axon_reset()
    except Exception:
        pass


def _run(seq, begin, end, trace=False):
    seq = np.asarray(seq)
    begin = np.asarray(begin).astype(np.int64)
    end = np.asarray(end).astype(np.int64)
    spans = end - begin
    R, NSLOT, pieces = _plan(begin, end)
    key = (R, NSLOT)
    if key not in _nc_cache:
        _nc_cache[key] = _build_nc(R, NSLOT)
    in_maps = _make_in_maps(seq, pieces, spans, R, NSLOT)
    try:
        res = run_bass_kernel_spmd(
            _nc_cache[key], in_maps, list(range(NCORES)), trace=trace
        )
    except Exception:
        _axon_reset()
        res = run_bass_kernel_spmd(
            _nc_cache[key], in_maps, list(range(NCORES)), trace=trace
        )
    out = np.zeros((B, D), dtype=np.float32)
    for ci in range(NCORES):
        part = res.results[ci]["out"]
        for s, (i, sb, se, ls) in enumerate(pieces[ci]):
            out[i] += part[s]
    return out, res


def kernel(seq, begin, end):
    out, _ = _run(seq, begin, end, trace=False)
    return out


# revision 36
# speedup vs baseline: 1.5136x; 1.0591x over previous
"""Ragged segment mean kernel for Trainium2 (8 NeuronCores, data-parallel).

Problem: seq [64, 2048, 1024] f32, begin/end [64] i64.
Output: out[i] = mean(seq[i, begin[i]:end[i], :])  -> [64, 1024] f32.

Strategy: dense-stream architecture. The host concatenates exactly the
segment rows of all samples into one global row stream, cuts it into 8
equal per-core chunks of R rows (R = ceil(total/8) rounded to 128), and
hands each core a contiguous [R, 1024] buffer plus a per-row slot id.
All device-side DMA offsets are then compile-time constants: no runtime
offset registers, no over-read, and per-core load is balanced to the
row.

The seq tensor is declared float32r end-to-end (same 4-byte storage,
host passes raw fp32 bits): the PE's f32r mode rounds operands
internally (~1e-4 rel, far inside the 2e-2 gate) and streams 1
cycle/row instead of fp32's 4, so no exact hi/resid two-pass split is
needed. The DVE expands the slot ids into a one-hot f32r routing mask
(16 is_equal sweeps) while the first data group is in flight; each
128-row chunk is then reduced on the PE as
acc[NSLOT, 512] += mask[128, NSLOT].T @ chunk[128, 512], accumulated
in PSUM across all chunks, scaled by 1/span, and stored. The kernel is
purely HBM-DMA-bound: 2 MiB groups split across both HWDGE rings
sustain ~420 GB/s/core (SDMA port line rate).

A sample whose rows straddle a core boundary becomes one piece per
core; each piece is scaled by the full 1/span on device and the host
adds the partial outputs while scattering back to batch order.
"""

import ml_dtypes
import numpy as np

import concourse.bacc as bacc
import concourse.bass as bass
import concourse.mybir as mybir
import concourse.tile as tile
from concourse.bass_utils import run_bass_kernel_spmd

B, L, D = 64, 2048, 1024
NCORES = 8
FREE = 512              # PSUM bank limit for matmul free dim
NMM = D // FREE         # 2 matmuls per 128-row chunk
GROUP = 512             # bulk rows per dma_start -> 2 MiB
NSLOT_DEFAULT = 16      # routing-mask columns (pieces per core)

_nc_cache = {}


def _group_sizes(R):
    """Split R rows into dma_start group sizes: 2 MiB bulk groups (16 KiB
    contiguous per partition each) with a tapered 256/128-row tail so
    group semaphores arrive at a fine cadence near the end and the PE
    drains right behind the stream. Groups stay smallish because a matmul
    can only start once its whole group's DMA semaphore fires: 4 MiB
    groups were measured to turn the PE into a bursty straggler that
    outlives the stream by >10us."""
    sizes = []
    rem = R
    while rem >= GROUP + 384:
        sizes.append(GROUP)
        rem -= GROUP
    if rem > 384:
        sizes.append(rem - 384)
        rem = 384
    if rem == 384:
        sizes += [256, 128]
    elif rem:
        sizes.append(rem)
    return sizes


def _ring_plan(sizes, mask_rows):
    """Assign each group to one of the two HWDGE rings (0=SP carries the
    mask first, 1=ACT), greedily balancing bytes so both rings finish
    together; a lopsided split leaves one ring streaming alone at half
    the aggregate rate at the end while the PE waits."""
    loads = [mask_rows, 0.0]
    rings = []
    for u in sizes:
        r = 0 if loads[0] <= loads[1] else 1
        rings.append(r)
        loads[r] += u
    return rings


def _build_nc(R, NSLOT):
    nc = bacc.Bacc("TRN2", target_bir_lowering=False)
    f32 = mybir.dt.float32
    NCH = R // 128
    i32 = mybir.dt.int32
    bf16 = mybir.dt.bfloat16
    # seq is staged by the host as bf16: the 2e-2 accuracy gate leaves
    # ~20x margin over bf16 rounding (~1e-3 on the segment mean), and
    # halving the bytes halves the HBM-bound stream time. bf16 matmuls
    # run 1 cycle/row like f32r.
    seqc = nc.dram_tensor("seqc", [R, D], bf16, kind="ExternalInput")
    # per-row slot ids; the 0/1 routing mask is expanded on-device (DVE
    # is_equal) so only NCH*4 bytes/partition ride the DMA ring instead
    # of the NCH*NSLOT*4-byte one-hot mask.
    rowslot = nc.dram_tensor("rowslot", [128, NCH], i32, kind="ExternalInput")
    invc = nc.dram_tensor("invc", [NSLOT, 1], f32, kind="ExternalInput")
    out = nc.dram_tensor("out", [NSLOT, D], f32, kind="ExternalOutput")

    sizes = _group_sizes(R)
    GF = (GROUP // 128) * D  # free size of a full group tile

    with tile.TileContext(nc) as tc:
        with (
            tc.tile_pool(name="const", bufs=1) as cpool,
            # one buffer per group: no rotation dependency, so the DMA
            # stream free-runs and the PE may consume groups in any order
            tc.tile_pool(name="seqp", bufs=len(sizes)) as spool,
            tc.tile_pool(name="accp", bufs=1, space="PSUM") as ppool,
            tc.tile_pool(name="resp", bufs=1) as rpool,
        ):
            rs = cpool.tile([128, NCH], i32, tag="rs")
            mt = cpool.tile([128, NCH * NSLOT], bf16)
            iv = cpool.tile([NSLOT, 1], f32)
            iv2 = cpool.tile([NSLOT, 1], f32, tag="iv2")
            warm = ppool.tile([NSLOT, NSLOT], f32, tag="warm")
            acc = ppool.tile([NSLOT, D], f32)

            # Slot ids head the ACT ring (tiny, lands in ~1us) while the
            # SP ring starts the seq stream immediately; the DVE expands
            # them into the one-hot f32r routing mask while the first
            # group is still in flight.
            nc.scalar.dma_start(out=rs[:], in_=rowslot[:])
            nc.scalar.dma_start(out=iv[:], in_=invc[:])
            nc.vector.tensor_copy(out=iv2[:], in_=iv[:])
            mtv = mt[:].rearrange("p (c s) -> p c s", s=NSLOT)
            for s in range(NSLOT):
                nc.vector.tensor_scalar(
                    out=mtv[:, :, s : s + 1],
                    in0=rs[:],
                    scalar1=s,
                    scalar2=None,
                    op0=mybir.AluOpType.is_equal,
                )

            # Issue every group DMA up front (stream order, alternating
            # rings); group tile[p, j*D+d] holds stream row g0 + p*J + j,
            # so partition p reads J*2KiB contiguous from DRAM.
            tiles = []
            g0 = 0
            ch = 0
            for gi, U in enumerate(sizes):
                J = U // 128
                t = spool.tile([128, GF], bf16)
                src = seqc[g0 : g0 + U, :].rearrange("(p j) d -> p (j d)", p=128)
                ring = nc.sync if gi % 2 == 0 else nc.scalar
                ring.dma_start(out=t[:, 0 : J * D], in_=src)
                tiles.append((t, ch, J))
                ch += J
                g0 += U
            # warmup matmul consuming only the mask tile: absorbs the
            # mask-arrival dependency into the PE clock so the first real
            # matmul waits only on the seq pipeline.
            nc.tensor.matmul(
                out=warm[:],
                lhsT=mt[:, 0:NSLOT],
                rhs=mt[:, 0:NSLOT],
                start=True,
                stop=True,
            )
            # PE consumption order: group DMA completion semaphores lag
            # their data by several us while the HBM stream saturates the
            # fabric, so the PE would idle out the lag before the final
            # group and then drain a backlog past the stream end. PSUM
            # accumulation is order-independent: process group 0
            # second-to-last so the pre-final-semaphore idle window is
            # filled with long-ready work, and keep the tiny tail group
            # last so the post-semaphore remainder is 2 matmuls.
            n = len(tiles)
            order = list(range(1, n - 1)) + [0, n - 1] if n >= 3 else list(range(n))
            for pi, gi in enumerate(order):
                t, ch, J = tiles[gi]
                for j in range(J):
                    lhs = mt[:, (ch + j) * NSLOT : (ch + j + 1) * NSLOT]
                    for h in range(NMM):
                        nc.tensor.matmul(
                            out=acc[:, h * FREE : (h + 1) * FREE],
                            lhsT=lhs,
                            rhs=t[:, j * D + h * FREE : j * D + (h + 1) * FREE],
                            start=(pi == 0 and j == 0),
                            stop=(pi == n - 1 and j == J - 1),
                        )

            res = rpool.tile([NSLOT, D], f32)
            nc.vector.tensor_scalar_mul(out=res[:], in0=acc[:], scalar1=iv2[:])
            nc.sync.dma_start(out=out[:], in_=res[:])
    nc.compile()
    return nc


def _plan(begin, end):
    """Order samples (big/small interleave to bound pieces per core),
    cut the global segment-row stream into 8 R-row cores, and return
    (R, NSLOT, pieces) with pieces[ci] = [(sample, src_b, src_e, local
    row start), ...]."""
    spans = (end - begin).astype(np.int64)
    desc = np.argsort(-spans, kind="stable")
    order = np.empty(B, dtype=np.int64)
    order[0::2] = desc[: (B + 1) // 2]
    order[1::2] = desc[(B + 1) // 2 :][::-1]
    total = int(spans.sum())
    per_core = -(-total // NCORES)          # ceil(total / 8)
    R = -(-per_core // 128) * 128           # round up to 128 rows

    pieces = [[] for _ in range(NCORES)]
    g = 0  # global stream cursor
    for i in order:
        b, e = int(begin[i]), int(end[i])
        while b < e:
            ci = g // R
            room = (ci + 1) * R - g
            n = min(e - b, room)
            pieces[ci].append((int(i), b, b + n, g - ci * R))
            b += n
            g += n
    nslot = max(NSLOT_DEFAULT, max(len(p) for p in pieces))
    nslot = -(-nslot // 8) * 8
    return R, nslot, pieces


def _make_in_maps(seq, pieces, spans, R, NSLOT):
    NCH = R // 128
    in_maps = []
    for ci in range(NCORES):
        seqc = np.zeros((R, D), dtype=ml_dtypes.bfloat16)
        row_slot = np.full(R, -1, dtype=np.int64)
        inv = np.zeros((NSLOT, 1), dtype=np.float32)
        for s, (i, sb, se, ls) in enumerate(pieces[ci]):
            n = se - sb
            seqc[ls : ls + n] = seq[i, sb:se]
            row_slot[ls : ls + n] = s
            inv[s, 0] = np.float32(1.0 / float(spans[i]))
        rs = np.full((128, NCH), -1, dtype=np.int32)
        g0 = 0
        chb = 0
        for U in _group_sizes(R):
            J = U // 128
            for j in range(J):
                rs[:, chb + j] = row_slot[g0 + np.arange(128) * J + j]
            g0 += U
            chb += J
        in_maps.append({"seqc": seqc, "rowslot": rs, "invc": inv})
    return in_maps


def _axon_reset():
    """Best-effort NeuronCore reset (recovers a device wedged by an
    earlier failed run in the same container)."""
    try:
        import ctypes

        import jax

        jax.devices()
        lib = ctypes.CDLL("/opt/axon/libaxon_pjrt.so")
        lib.axon_reset.restype = ctypes.c_int64
        lib.axon_reset()
    except Exception:
        pass


def _run(seq, begin, end, trace=False):
    seq = np.asarray(seq)
    begin = np.asarray(begin).astype(np.int64)
    end = np.asarray(end).astype(np.int64)
    spans = end - begin
    R, NSLOT, pieces = _plan(begin, end)
    key = (R, NSLOT)
    if key not in _nc_cache:
        _nc_cache[key] = _build_nc(R, NSLOT)
    in_maps = _make_in_maps(seq, pieces, spans, R, NSLOT)
    try:
        res = run_bass_kernel_spmd(
            _nc_cache[key], in_maps, list(range(NCORES)), trace=trace
        )
    except Exception:
        _axon_reset()
        res = run_bass_kernel_spmd(
            _nc_cache[key], in_maps, list(range(NCORES)), trace=trace
        )
    out = np.zeros((B, D), dtype=np.float32)
    for ci in range(NCORES):
        part = res.results[ci]["out"]
        for s, (i, sb, se, ls) in enumerate(pieces[ci]):
            out[i] += part[s]
    return out, res


def kernel(seq, begin, end):
    out, _ = _run(seq, begin, end, trace=False)
    return out
